# revision 1
# baseline (speedup 1.0000x reference)
"""GroupNorm + single-head self-attention block (B=16, C=512, H=W=32) on 8
TRN2 NeuronCores.

Sharding: pure data-parallel over batch — 2 samples per core, no collectives.

Per-sample dataflow (C=512 channels, N=1024 pixels), everything laid out
channels-on-partitions so no transposes are ever needed:

  x   [c, n]   4 tiles [128, 1024]
  GN: per-channel mean/var via bn_stats, group (16-ch) aggregation via a
      tiny matmul against a group-indicator matrix, scatter back the same
      way; h = a_c * x - b_c  (per-partition scalars).
  Q = wq @ h -> [o, n]  (lhsT = wqT tiles, rhs = h)          + bq
  K = wk @ h -> [o, m]                                        + bk
  V [m, c] = h_tile.T @ wvT   (h as the stationary operand -> V lands
      pixel-major; bias bv folded into c0 = wo @ bv + bo at the end)
  ST [m, n] = K_tile.T @ Q    (scores, transposed layout)
  E = exp(ST / sqrt(C))       (no max-subtraction: scores are O(5))
  R [128, n] = 1 / (ones128.T @ E)   (softmax denominators, accumulated
      directly in broadcast form on the PE; one reciprocal per chunk)
  OT [c, n] = V_tile.T @ E    (un-normalized attn output, channel-major)
  OT *= R                     (rides the PSUM evacuation)
  o2 [o, n] = woT_tile.T @ OT
  y = x + o2 + c0[o]          (single fused scalar_tensor_tensor)

Matmuls run in bf16 (fp32 PSUM accumulation; CoreSim-validated rel err
3.5e-4 vs the fp32 reference). Emission is phase-major across the two
samples so one sample's matmuls fill the other's dependency stalls;
small constants ride a single packed DMA; weights load behind x on the
HWDGE queue; outputs drain via the GPSIMD SWDGE path.
"""

import numpy as np

import concourse.bass as bass
import concourse.mybir as mybir
from concourse import tile
from concourse.bass_utils import run_bass_kernel_spmd


def _install_drain_patch():
    """This walrus build rejects Drain instructions carrying more than one
    semaphore wait (setupSyncWait<CTRL_NO_STRUCT>). Split the TileContext
    tail drain's waits across a chain of single-wait drains."""
    import concourse.tile as tile_mod
    from concourse.vector_clock import ScopedClock

    if getattr(tile_mod.TileContext, "_drain_patch_installed", False):
        return

    def _patched(self, tick_clock, wait_clock):
        nc = self.nc
        drain_inst = nc.sync.drain()
        wait_clock.add_sem_waits(
            drain_inst.ins, ScopedClock({None: tick_clock.global_clock})
        )
        si = drain_inst.ins.sync_info
        waits = list(si.on_wait or []) if si is not None else []
        if len(waits) > 1:
            si.on_wait = waits[:1]
            for w in waits[1:]:
                extra = nc.sync.drain()
                extra.ins.sync_info = mybir.SyncInfo(on_wait=[w], on_update=[])

        nc.all_engine_barrier()
        assert self.sems is not None
        popped = nc._tile_sem_poison_stack.pop()
        assert popped is self._sem_poison
        nc.clear_and_free_semaphores(list(self.sems.allocated().values()))
        nc.all_engine_barrier()

    tile_mod.TileContext._drain_and_barrier = _patched
    tile_mod.TileContext._drain_patch_installed = True


_install_drain_patch()

F32 = mybir.dt.float32
F32R = mybir.dt.float32r
BF16 = mybir.dt.bfloat16

B, C, H, W = 16, 512, 32, 32
N = H * W                      # 1024 pixels
NCORES = 8
S = B // NCORES                # samples per core
CT = C // 128                  # 4 channel tiles
NW = 512                       # matmul moving-operand chunk (fp32r max)
NCH = N // NW                  # 2 chunks
MT = N // 128                  # 8 pixel tiles
GROUPS = 32
GSIZE = C // GROUPS            # 16 channels per group
GPT = 128 // GSIZE             # 8 groups per channel tile
EPS = 1e-5

COMPUTE = "bf16"               # "f32r" | "bf16" | "f32"

CDT = {"bf16": BF16, "f32r": F32R, "f32": F32}[COMPUTE]


def _cc(ap):
    """Cast an AP for TensorEngine consumption (tiles already carry the
    compute dtype; this is now a no-op kept for clarity)."""
    return ap


def _split_waits(nc, maxw=1):
    """This walrus build caps the number of sync waits an instruction can
    carry (varies by instruction class; Drain and Matmult/LDWEIGHTS observed
    failing). Hoist excess waits onto standalone EventSemaphore instructions
    inserted just before, on the same engine."""
    cnt = 0
    for f in nc.m.functions:
        for bb in f.blocks:
            insts = list(bb.instructions)
            out = []
            changed = False
            for inst in insts:
                si = inst.sync_info
                waits = list(si.on_wait) if (si is not None and si.on_wait) else []
                if len(waits) > maxw:
                    for w in waits[:-maxw]:
                        ev = mybir.InstEventSemaphore(
                            name=f"waitsplit_{cnt}", ins=[], outs=[])
                        cnt += 1
                        ev.engine = inst.engine
                        ev.sync_info = mybir.SyncInfo(on_wait=[w], on_update=[])
                        out.append(ev)
                    si.on_wait = waits[-maxw:]
                    changed = True
                out.append(inst)
            if changed:
                _replace_block_instructions(bb, out)
    return cnt


def _replace_block_instructions(bb, insts):
    try:
        bb.instructions = insts
        return
    except Exception:
        pass
    try:
        bb.instructions.clear()
        for i in insts:
            bb.instructions.append(i)
        return
    except Exception:
        pass
    raise RuntimeError("cannot rewrite block instructions")


def build_nc(split_waits=True):
    nc = bass.Bass(target_bir_lowering=False)

    x_ext = nc.declare_dram_parameter("x", [S, CT, 128, N], F32, isOutput=False)
    w_ext = {}
    for w in ("wq", "wk", "wv", "wo"):
        w_ext[w] = nc.declare_dram_parameter(w, [CT, 128, C], CDT, isOutput=False)
    b_ext = {}
    # cblob columns: bq[4] bk[4] c0[4] gnw[4] gnb[4] gmat[8] -> [128, 28] f32
    b_ext["cblob"] = nc.declare_dram_parameter("cblob", [128, 28], F32,
                                               isOutput=False)
    b_ext["gmt"] = nc.declare_dram_parameter("gmt", [GPT, 128], F32,
                                             isOutput=False)
    b_ext["ones2"] = nc.declare_dram_parameter("ones2", [128, 128], CDT,
                                               isOutput=False)
    out_ext = nc.declare_dram_parameter("out", [S, CT, 128, N], F32, isOutput=True)

    with tile.TileContext(nc) as tc:
        _body(nc, tc, x_ext, w_ext, b_ext, out_ext)
    if split_waits:
        _split_waits(nc)
    return nc


def _body(nc, tc, x_ext, w_ext, b_ext, out_ext):
    import contextlib

    ctx = contextlib.ExitStack()
    with ctx:
        consts = ctx.enter_context(tc.tile_pool(name="consts", bufs=1))
        sb = ctx.enter_context(tc.tile_pool(name="sb", bufs=1))
        ps = ctx.enter_context(tc.tile_pool(name="ps", space="PSUM", bufs=1))

        # ---------------- constants ----------------
        # Weight tiles are allocated now but their DMAs are emitted after the
        # x loads (phase_weights) so x wins the head-of-line on HWDGE.
        w_sb = {}
        for w in ("wq", "wk", "wv", "wo"):
            w_sb[w] = [
                consts.tile([128, C], CDT, name=f"{w}_{ct}", tag=f"{w}_{ct}")
                for ct in range(CT)
            ]

        def phase_weights(names):
            for w in names:
                for ct in range(CT):
                    nc.sync.dma_start(out=w_sb[w][ct], in_=w_ext[w][ct])
        cblob = consts.tile([128, 28], F32, tag="cblob")
        nc.gpsimd.dma_start(out=cblob, in_=b_ext["cblob"][:, :])
        b_sb = {}
        for bi, b in enumerate(("bq", "bk", "c0", "gnw", "gnb")):
            b_sb[b] = [cblob[:, bi * CT + ct:bi * CT + ct + 1]
                       for ct in range(CT)]

        ones2 = consts.tile([128, 128], CDT, tag="ones2")
        nc.gpsimd.dma_start(out=ones2, in_=b_ext["ones2"][:, :])
        warm = ps.tile([128, NW], F32, tag="small", bufs=2)
        for wi in range(12):
            nc.tensor.matmul(warm[:, 0:128], ones2, ones2,
                             start=(wi == 0), stop=(wi == 11))

        # Group-indicator matrices.
        # gmat[c, g] = 1/GSIZE where c // GSIZE == g   (gather:  [128, GPT])
        # gmt [g, c] = 1       where c // GSIZE == g   (scatter: [GPT, 128])
        gmat = cblob[:, 20:20 + GPT]
        gmt = consts.tile([GPT, 128], F32, tag="gmt")
        nc.gpsimd.dma_start(out=gmt, in_=b_ext["gmt"][:, :])

        eps_g = consts.tile([GPT, 1], F32, tag="eps_g")
        nc.vector.memset(eps_g, EPS)

        # c0[o] = (wo @ bv)[o] + bo[o], precomputed on the host — folds the
        # V bias exactly: after softmax-normalization the bv term contributes
        # bv broadcast through wo (attention rows sum to 1).
        c0_sb = b_sb["c0"]

        # ---------------- per-sample pipelines, emitted phase-major ----------------
        # Emitting each phase for both samples back-to-back lets the PE fill
        # one sample's dependency stalls (GroupNorm chain, softmax-denominator
        # chain) with the other sample's matmuls.
        inv_sqrt_c = 1.0 / float(np.sqrt(C))
        st = [dict() for _ in range(S)]

        def phase_load(s):
            x_sb = []
            for ct in range(CT):
                xt = sb.tile([128, N], F32, name=f"x{s}_{ct}", tag=f"x_{ct}",
                             bufs=2)
                for q in range(2):
                    nc.sync.dma_start(out=xt[:, q * 512:(q + 1) * 512],
                                      in_=x_ext[s, ct, :, q * 512:(q + 1) * 512])
                x_sb.append(xt)
            st[s]["x"] = x_sb

        def phase_gn(s):
            x_sb = st[s]["x"]
            stats3 = []
            for ct in range(CT):
                s3 = sb.tile([128, 3], F32, tag=f"s3_{ct}", bufs=2)
                if ct < 2:
                    # DVE path: bn_stats -> [mean, var], plus mean^2
                    st6 = sb.tile([128, 2, 6], F32, tag="st6", bufs=4)
                    nc.vector.bn_stats(out=st6[:, 0, :], in_=x_sb[ct][:, 0:512])
                    nc.vector.bn_stats(out=st6[:, 1, :],
                                       in_=x_sb[ct][:, 512:1024])
                    nc.vector.bn_aggr(out=s3[:, 0:2], in_=st6)
                    nc.vector.tensor_mul(out=s3[:, 2:3], in0=s3[:, 0:1],
                                         in1=s3[:, 0:1])
                else:
                    # ACT path: accum_out sums along the free axis.
                    # col0 = mean (scale 1/N), col1 = E[x^2] ((x/sqrt(N))^2),
                    # col2 = 0.  Downstream uses col1+col2 = E[x^2], same as
                    # var + mean^2 on the DVE path.
                    scr = sb.tile([128, N], CDT, tag="gnscr", bufs=2)
                    nc.scalar.activation(
                        out=scr, in_=x_sb[ct],
                        func=mybir.ActivationFunctionType.Copy,
                        scale=1.0 / N, accum_out=s3[:, 0:1])
                    nc.scalar.activation(
                        out=scr, in_=x_sb[ct],
                        func=mybir.ActivationFunctionType.Square,
                        scale=1.0 / float(np.sqrt(N)), accum_out=s3[:, 1:2])
                    nc.vector.memset(s3[:, 2:3], 0.0)
                stats3.append(s3)

            h_sb = [None] * CT
            for ct in range(CT):
                gp = ps.tile([GPT, 3], F32, tag="small", bufs=2)
                nc.tensor.matmul(gp, gmat, stats3[ct], start=True, stop=True)
                gs = sb.tile([GPT, 3], F32, tag="gs", bufs=4)
                nc.vector.tensor_copy(out=gs, in_=gp)
                # var_g = (E[var] + E[mean^2]) - (E[mean])^2
                m2 = sb.tile([GPT, 3], F32, tag="m2", bufs=4)
                nc.vector.tensor_add(out=m2[:, 1:2], in0=gs[:, 1:2],
                                     in1=gs[:, 2:3])
                nc.vector.tensor_mul(out=m2[:, 0:1], in0=gs[:, 0:1],
                                     in1=gs[:, 0:1])
                nc.vector.tensor_sub(out=m2[:, 2:3], in0=m2[:, 1:2],
                                     in1=m2[:, 0:1])
                s2 = sb.tile([GPT, 2], F32, tag="s2", bufs=4)
                nc.scalar.activation(out=s2[:, 1:2], in_=m2[:, 2:3],
                                     func=mybir.ActivationFunctionType.Sqrt,
                                     bias=eps_g, scale=1.0)
                nc.gpsimd.tensor_copy(out=s2[:, 0:1], in_=gs[:, 0:1])
                nc.vector.reciprocal(out=s2[:, 1:2], in_=s2[:, 1:2])

                abp = ps.tile([128, 2], F32, tag="small", bufs=2)
                nc.tensor.matmul(abp, gmt, s2, start=True, stop=True)
                a_c = sb.tile([128, 1], F32, tag=f"a_{ct}", bufs=2)
                nc.vector.tensor_mul(out=a_c, in0=abp[:, 1:2],
                                     in1=b_sb["gnw"][ct])
                bneg = sb.tile([128, 1], F32, tag=f"bneg_{ct}", bufs=2)
                nc.vector.scalar_tensor_tensor(
                    out=bneg, in0=abp[:, 0:1], scalar=a_c,
                    in1=b_sb["gnb"][ct],
                    op0=mybir.AluOpType.mult, op1=mybir.AluOpType.subtract,
                )
                ht = sb.tile([128, N], CDT, name=f"h{s}_{ct}", tag=f"h_{ct}",
                             bufs=2)
                eng = nc.vector if ct < 2 else nc.gpsimd
                eng.tensor_scalar(
                    out=ht, in0=x_sb[ct], scalar1=a_c, scalar2=bneg,
                    op0=mybir.AluOpType.mult, op1=mybir.AluOpType.subtract,
                )
                h_sb[ct] = ht
            st[s]["h"] = h_sb

        def phase_qkv(s):
            h_sb = st[s]["h"]
            q_sb, k_sb = [], []
            for name, wt, bias, dst in (("q", "wq", "bq", q_sb),
                                        ("k", "wk", "bk", k_sb)):
                for ot in range(CT):
                    t = sb.tile([128, N], CDT, name=f"{name}{s}_{ot}",
                                tag=f"{name}_{ot}", bufs=2)
                    dst.append(t)
                    for nch in range(NCH):
                        pp = ps.tile([128, NW], F32, tag="mm", bufs=6)
                        for ct in range(CT):
                            nc.tensor.matmul(
                                pp,
                                _cc(w_sb[wt][ct][:, ot * 128:(ot + 1) * 128]),
                                _cc(h_sb[ct][:, nch * NW:(nch + 1) * NW]),
                                start=(ct == 0), stop=(ct == CT - 1),
                            )
                        nc.vector.tensor_scalar_add(
                            out=t[:, nch * NW:(nch + 1) * NW], in0=pp,
                            scalar1=b_sb[bias][ot],
                        )
            v_sb = []
            for mt in range(MT):
                vt = sb.tile([128, C], CDT, name=f"v{s}_{mt}", tag=f"v_{mt}",
                             bufs=2)
                vp = ps.tile([128, NW], F32, tag="mm", bufs=6)
                for ct in range(CT):
                    nc.tensor.matmul(
                        vp,
                        _cc(h_sb[ct][:, mt * 128:(mt + 1) * 128]),
                        _cc(w_sb["wv"][ct]),
                        start=(ct == 0), stop=(ct == CT - 1),
                    )
                nc.scalar.copy(out=vt, in_=vp)
                v_sb.append(vt)
            st[s]["q"], st[s]["k"], st[s]["v"] = q_sb, k_sb, v_sb

        def phase_st(s):
            q_sb, k_sb = st[s]["q"], st[s]["k"]
            e_sb = [sb.tile([128, N], CDT, name=f"e{s}_{mt}", tag=f"e_{mt}",
                            bufs=2) for mt in range(MT)]
            # nch outer: all of chunk 0's scores+exp land first, so the
            # softmax-denominator accumulation for chunk 0 overlaps chunk 1.
            for nch in range(NCH):
                for mt in range(MT):
                    sp = ps.tile([128, NW], F32, tag="mm", bufs=6)
                    for ct in range(CT):
                        nc.tensor.matmul(
                            sp,
                            _cc(k_sb[ct][:, mt * 128:(mt + 1) * 128]),
                            _cc(q_sb[ct][:, nch * NW:(nch + 1) * NW]),
                            start=(ct == 0), stop=(ct == CT - 1),
                        )
                    nc.scalar.activation(
                        out=e_sb[mt][:, nch * NW:(nch + 1) * NW], in_=sp,
                        func=mybir.ActivationFunctionType.Exp,
                        scale=inv_sqrt_c,
                    )
            st[s]["e"] = e_sb

        def phase_sr(s):
            e_sb = st[s]["e"]
            # R[p, n] = 1 / sum_m E[m, n], built directly in broadcast form:
            # ones2.T @ E accumulates the column sums into every partition.
            R_sb = sb.tile([128, N], F32, tag="R", bufs=2)
            for nch in range(NCH):
                srp = ps.tile([128, NW], F32, tag="small", bufs=2)
                for mt in range(MT):
                    nc.tensor.matmul(
                        srp, _cc(ones2),
                        _cc(e_sb[mt][:, nch * NW:(nch + 1) * NW]),
                        start=(mt == 0), stop=(mt == MT - 1),
                    )
                nc.vector.reciprocal(out=R_sb[:, nch * NW:(nch + 1) * NW],
                                     in_=srp)
            st[s]["R"] = R_sb

        def phase_ot(s):
            v_sb, e_sb, R_sb = st[s]["v"], st[s]["e"], st[s]["R"]
            ot_sb = [sb.tile([128, N], CDT, name=f"ot{s}_{ct}", tag=f"ot_{ct}",
                             bufs=2) for ct in range(CT)]
            # nch outer: chunk 0's four OT tiles finish first, so the o2
            # projection for chunk 0 starts half a phase earlier.
            for nch in range(NCH):
                for ct in range(CT):
                    op_ = ps.tile([128, NW], F32, tag="mm", bufs=6)
                    for mt in range(MT):
                        nc.tensor.matmul(
                            op_,
                            _cc(v_sb[mt][:, ct * 128:(ct + 1) * 128]),
                            _cc(e_sb[mt][:, nch * NW:(nch + 1) * NW]),
                            start=(mt == 0), stop=(mt == MT - 1),
                        )
                    nc.vector.tensor_mul(
                        out=ot_sb[ct][:, nch * NW:(nch + 1) * NW], in0=op_,
                        in1=R_sb[:, nch * NW:(nch + 1) * NW],
                    )
            st[s]["ot"] = ot_sb

        def phase_o2(s):
            x_sb, ot_sb = st[s]["x"], st[s]["ot"]
            for nch in range(NCH):
                for ot in range(CT):
                    o2p = ps.tile([128, NW], F32, tag="mm", bufs=6)
                    for ct in range(CT):
                        nc.tensor.matmul(
                            o2p,
                            _cc(w_sb["wo"][ct][:, ot * 128:(ot + 1) * 128]),
                            _cc(ot_sb[ct][:, nch * NW:(nch + 1) * NW]),
                            start=(ct == 0), stop=(ct == CT - 1),
                        )
                    # y = (o2 + c0) + x, written in place over x
                    nc.vector.scalar_tensor_tensor(
                        out=x_sb[ot][:, nch * NW:(nch + 1) * NW], in0=o2p,
                        scalar=c0_sb[ot],
                        in1=x_sb[ot][:, nch * NW:(nch + 1) * NW],
                        op0=mybir.AluOpType.add, op1=mybir.AluOpType.add,
                    )
                    nc.gpsimd.dma_start(
                        out=out_ext[s, ot, :, nch * NW:(nch + 1) * NW],
                        in_=x_sb[ot][:, nch * NW:(nch + 1) * NW])

        # x(s0) first (feeds GroupNorm), then the weights QKV needs first,
        # then x(s1), then the rest — keeps the first QKV LDWEIGHTS fed.
        phase_load(0)
        phase_weights(("wq", "wk"))
        phase_load(1)
        phase_weights(("wv", "wo"))
        for phase in (phase_gn, phase_qkv, phase_st, phase_sr):
            for s in range(S):
                phase(s)
        for s in range(S):
            phase_ot(s)
            phase_o2(s)


_CACHE = {}


def make_in_maps(inputs):
    """Host-side sharding/layout prep shared by kernel() and the test/sim
    harnesses."""
    x = np.asarray(inputs["x"], dtype=np.float32)
    assert x.shape == (B, C, H, W)

    if COMPUTE == "bf16":
        import ml_dtypes
        wdt = ml_dtypes.bfloat16
    else:
        wdt = np.float32

    def wprep(w):
        # [o, c] -> transpose to [c, o] -> tile rows of 128 channels
        return np.ascontiguousarray(
            np.asarray(w, dtype=np.float32).T.reshape(CT, 128, C)
        ).astype(wdt)

    c0 = (np.asarray(inputs["wo"], dtype=np.float64)
          @ np.asarray(inputs["bv"], dtype=np.float64)
          + np.asarray(inputs["bo"], dtype=np.float64)).astype(np.float32)
    base = {
        "wq": wprep(inputs["wq"]), "wk": wprep(inputs["wk"]),
        "wv": wprep(inputs["wv"]), "wo": wprep(inputs["wo"]),
    }
    gmat = np.zeros((128, GPT), dtype=np.float32)
    gmt = np.zeros((GPT, 128), dtype=np.float32)
    for g in range(GPT):
        gmat[g * GSIZE:(g + 1) * GSIZE, g] = 1.0 / GSIZE
        gmt[g, g * GSIZE:(g + 1) * GSIZE] = 1.0
    cblob = np.zeros((128, 28), dtype=np.float32)
    for bi, arr in enumerate((inputs["bq"], inputs["bk"], c0,
                              inputs["gn_weight"], inputs["gn_bias"])):
        cblob[:, bi * CT:(bi + 1) * CT] = np.asarray(
            arr, dtype=np.float32).reshape(CT, 128).T
    cblob[:, 20:20 + GPT] = gmat
    base["cblob"] = cblob
    base["gmt"] = gmt
    base["ones2"] = np.ones((128, 128), dtype=wdt)
    xr = x.reshape(NCORES, S, CT, 128, N)
    return [dict(base, x=np.ascontiguousarray(xr[i])) for i in range(NCORES)]


def kernel(**inputs):
    if "nc" not in _CACHE:
        _CACHE["nc"] = build_nc()
    nc = _CACHE["nc"]

    in_maps = make_in_maps(inputs)
    res = run_bass_kernel_spmd(nc, in_maps, core_ids=list(range(NCORES)))

    out = np.empty((NCORES, S, CT, 128, N), dtype=np.float32)
    for i in range(NCORES):
        out[i] = res.results[i]["out"]
    return out.reshape(B, C, H, W)



# revision 6
# speedup vs baseline: 2.3873x; 2.3873x over previous
"""GroupNorm + single-head self-attention block (B=16, C=512, H=W=32) on 8
TRN2 NeuronCores.

Sharding: pure data-parallel over batch - 2 samples per core, no collectives.

v2: fused-weight fp8 pipeline.  The four C*C projections collapse to two by
constant-folding on the host:

  M  = wq^T wk / 1          scores = h^T M h   (q/k projections fused)
  W2 = wo wv                o2     = attn^T (W2 h)   (v/o projections fused)

Per-sample dataflow (C=512 channels, N=1024 pixels), channels/pixels on
partitions, every matmul fp8e4 with perf_mode=DoubleRow (2 contraction
subtiles per instruction, 2 fp8 MACs/cell/cycle):

  x   [c, n]    4 tiles [128, 1024] f32
  GN: per-channel mean/var (bn_stats on DVE for 2 tiles, ACT accum for 2),
      16-ch group aggregation via tiny matmuls against group-indicator
      matrices; h = a'x - b'  (a', b' carry the fp8 scale S_h=16).
  T  [c2, n] = M~^T h    (DR pairs over c1-tiles)        -> fp8, scale 8
  V2 [m, o]  = h^T W2~   (h stationary, DR pairs c-tiles) -> fp8, scale 16
  ST [m, n]  = h^T T     (DR pairs c2-tiles)
  E = exp(ST/(sqrt(C)*S_h*S_T) - 1.5)  (shift cancels in softmax; keeps
      exp() inside fp8e4's 240 max)                       -> fp8, scale 1
  den[n] = (16*ones)^T E  (DR; the 16 bakes in 1/S_V2)
  R = 1/den  (DVE reciprocal)
  O  [o, n]  = V2^T E    (DR pairs m-tiles)  == unnormalized o2
  y = (O*R + c0) + x     (c0 = wo bv + bo host-folded; two DVE ops riding
      the PSUM evacuation, written in place over x)

Softmax-constant terms of the q/k biases cancel exactly; the surviving
term (wk^T bq)^T h is emitted as tiny extra matmuls only when bq/bk are
nonzero (the graph is built per bias-structure and cached).  All fp8
scales are powers of two folded into existing activation scales, so they
cost nothing.  Host-side prep is weight-folding + layout only.

Numerics (numpy emulation vs fp32 reference): rel err ~4.5e-3.
"""

import numpy as np

import concourse.bass as bass
import concourse.mybir as mybir
from concourse import tile
from concourse.bass_utils import run_bass_kernel_spmd


def _install_drain_patch():
    """This walrus build rejects Drain instructions carrying more than one
    semaphore wait (setupSyncWait<CTRL_NO_STRUCT>). Split the TileContext
    tail drain's waits across a chain of single-wait drains."""
    import concourse.tile as tile_mod
    from concourse.vector_clock import ScopedClock

    if getattr(tile_mod.TileContext, "_drain_patch_installed", False):
        return

    def _patched(self, tick_clock, wait_clock):
        nc = self.nc
        drain_inst = nc.sync.drain()
        wait_clock.add_sem_waits(
            drain_inst.ins, ScopedClock({None: tick_clock.global_clock})
        )
        si = drain_inst.ins.sync_info
        waits = list(si.on_wait or []) if si is not None else []
        if len(waits) > 1:
            si.on_wait = waits[:1]
            for w in waits[1:]:
                extra = nc.sync.drain()
                extra.ins.sync_info = mybir.SyncInfo(on_wait=[w], on_update=[])

        nc.all_engine_barrier()
        assert self.sems is not None
        popped = nc._tile_sem_poison_stack.pop()
        assert popped is self._sem_poison
        nc.clear_and_free_semaphores(list(self.sems.allocated().values()))
        nc.all_engine_barrier()

    tile_mod.TileContext._drain_and_barrier = _patched
    tile_mod.TileContext._drain_patch_installed = True


_install_drain_patch()

F32 = mybir.dt.float32
FP8 = mybir.dt.float8e4
DR = mybir.MatmulPerfMode.DoubleRow

B, C, H, W = 16, 512, 32, 32
N = H * W                      # 1024 pixels
NCORES = 8
S = B // NCORES                # samples per core
CT = C // 128                  # 4 channel tiles
NW = 512                       # psum chunk (one bank of fp32)
NCH = N // NW                  # 2 chunks
MT = N // 128                  # 8 pixel tiles
GROUPS = 32
GSIZE = C // GROUPS            # 16 channels per group
GPT = 128 // GSIZE             # 8 groups per channel tile
EPS = 1e-5

# fp8 scale plan (all powers of two; folded into existing scalars)
S_H = 16.0                     # h
S_M = 256.0                    # M~ = wq^T wk
S_T = 8.0                      # T
S_W2 = 256.0                   # W2~ = wo wv
S_V2 = 16.0                    # V2 (also baked into the den "ones")
EK = 1.5                       # exp shift, cancels in softmax
T_EVAC = S_T / (S_H * S_M)             # 2^-9
V2_EVAC = S_V2 / (S_H * S_W2)          # 2^-8
E_SCALE = 1.0 / (S_H * S_T * float(np.sqrt(C)))


def _split_waits(nc, maxw=1):
    """This walrus build caps the number of sync waits an instruction can
    carry. Hoist excess waits onto standalone EventSemaphore instructions
    inserted just before, on the same engine."""
    cnt = 0
    for f in nc.m.functions:
        for bb in f.blocks:
            insts = list(bb.instructions)
            out = []
            changed = False
            for inst in insts:
                si = inst.sync_info
                waits = list(si.on_wait) if (si is not None and si.on_wait) else []
                if len(waits) > maxw:
                    for w in waits[:-maxw]:
                        ev = mybir.InstEventSemaphore(
                            name=f"waitsplit_{cnt}", ins=[], outs=[])
                        cnt += 1
                        ev.engine = inst.engine
                        ev.sync_info = mybir.SyncInfo(on_wait=[w], on_update=[])
                        out.append(ev)
                    si.on_wait = waits[-maxw:]
                    changed = True
                out.append(inst)
            if changed:
                _replace_block_instructions(bb, out)
    return cnt


def _replace_block_instructions(bb, insts):
    try:
        bb.instructions = insts
        return
    except Exception:
        pass
    try:
        bb.instructions.clear()
        for i in insts:
            bb.instructions.append(i)
        return
    except Exception:
        pass
    raise RuntimeError("cannot rewrite block instructions")


def build_nc(has_qk_bias=False, split_waits=True):
    nc = bass.Bass(target_bir_lowering=False)

    x_ext = nc.declare_dram_parameter("x", [S, CT, 128, N], F32, isOutput=False)
    mfus_ext = nc.declare_dram_parameter("mfus", [128, CT, C], FP8, isOutput=False)
    w2fus_ext = nc.declare_dram_parameter("w2fus", [128, CT, C], FP8,
                                          isOutput=False)
    onesden_ext = nc.declare_dram_parameter("onesden", [128, 2 * 128], FP8,
                                            isOutput=False)
    # cblob columns: c0[4] gnw'[4] gnb'[4] gmat[8] -> [128, 20] f32
    cblob_ext = nc.declare_dram_parameter("cblob", [128, 20], F32,
                                          isOutput=False)
    gmt_ext = nc.declare_dram_parameter("gmt", [GPT, 128], F32, isOutput=False)
    rvec_ext = None
    if has_qk_bias:
        rvec_ext = nc.declare_dram_parameter("rvec", [128, CT, 1], FP8,
                                             isOutput=False)
    out_ext = nc.declare_dram_parameter("out", [S, CT, 128, N], F32,
                                        isOutput=True)

    with tile.TileContext(nc) as tc:
        _body(nc, tc, x_ext, mfus_ext, w2fus_ext, onesden_ext, cblob_ext,
              gmt_ext, rvec_ext, out_ext)
    if split_waits:
        _split_waits(nc)
    return nc


def _body(nc, tc, x_ext, mfus_ext, w2fus_ext, onesden_ext, cblob_ext,
          gmt_ext, rvec_ext, out_ext):
    import contextlib

    ctx = contextlib.ExitStack()
    with ctx:
        consts = ctx.enter_context(tc.tile_pool(name="consts", bufs=1))
        sb = ctx.enter_context(tc.tile_pool(name="sb", bufs=1))
        ps = ctx.enter_context(tc.tile_pool(name="ps", space="PSUM", bufs=1))

        # ---------------- constants ----------------
        mfus = consts.tile([128, CT, C], FP8, tag="mfus")
        w2fus = consts.tile([128, CT, C], FP8, tag="w2fus")
        onesden = consts.tile([128, 2, 128], FP8, tag="onesden")
        cblob = consts.tile([128, 20], F32, tag="cblob")
        gmt = consts.tile([GPT, 128], F32, tag="gmt")
        nc.gpsimd.dma_start(out=cblob, in_=cblob_ext[:, :])
        nc.gpsimd.dma_start(out=gmt, in_=gmt_ext[:, :])
        nc.gpsimd.dma_start(out=onesden[:, 0, :], in_=onesden_ext[:, 0:128])
        nc.gpsimd.dma_start(out=onesden[:, 1, :], in_=onesden_ext[:, 128:256])

        b_sb = {}
        for bi, b in enumerate(("c0", "gnw", "gnb")):
            b_sb[b] = [cblob[:, bi * CT + ct:bi * CT + ct + 1]
                       for ct in range(CT)]
        gmat = cblob[:, 12:12 + GPT]

        rvec = None
        if rvec_ext is not None:
            rvec = consts.tile([128, CT, 1], FP8, tag="rvec")
            nc.gpsimd.dma_start(out=rvec, in_=rvec_ext[:, :, :])

        eps_g = consts.tile([GPT, 1], F32, tag="eps_g")
        nc.vector.memset(eps_g, EPS)
        nek = consts.tile([128, 1], F32, tag="nek")
        nc.vector.memset(nek, -EK)

        def phase_weights():
            nc.sync.dma_start(out=mfus[:, 0:2, :], in_=mfus_ext[:, 0:2, :])
            nc.sync.dma_start(out=mfus[:, 2:4, :], in_=mfus_ext[:, 2:4, :])
            nc.sync.dma_start(out=w2fus[:, 0:2, :], in_=w2fus_ext[:, 0:2, :])
            nc.sync.dma_start(out=w2fus[:, 2:4, :], in_=w2fus_ext[:, 2:4, :])

        # PE warmup: spin the HAM up before the first real matmuls.
        warm = ps.tile([128, NW], F32, tag="small", bufs=2)
        for wi in range(12):
            nc.tensor.matmul(warm[:, 0:128], onesden[:, 0, :], onesden[:, 0, :],
                             start=(wi == 0), stop=(wi == 11))

        # ---------------- per-sample pipelines, emitted phase-major ----------------
        st = [dict() for _ in range(S)]

        def phase_load(s):
            x_sb = []
            for ct in range(CT):
                xt = sb.tile([128, N], F32, name=f"x{s}_{ct}", tag=f"x_{ct}",
                             bufs=2)
                for q in range(2):
                    nc.sync.dma_start(out=xt[:, q * 512:(q + 1) * 512],
                                      in_=x_ext[s, ct, :, q * 512:(q + 1) * 512])
                x_sb.append(xt)
            st[s]["x"] = x_sb

        def phase_gn(s):
            x_sb = st[s]["x"]
            stats3 = []
            for ct in range(CT):
                s3 = sb.tile([128, 3], F32, tag=f"s3_{ct}", bufs=2)
                if ct < 2:
                    # DVE path: bn_stats -> [mean, var], plus mean^2
                    st6 = sb.tile([128, 2, 6], F32, tag="st6", bufs=4)
                    nc.vector.bn_stats(out=st6[:, 0, :], in_=x_sb[ct][:, 0:512])
                    nc.vector.bn_stats(out=st6[:, 1, :],
                                       in_=x_sb[ct][:, 512:1024])
                    nc.vector.bn_aggr(out=s3[:, 0:2], in_=st6)
                    nc.vector.tensor_mul(out=s3[:, 2:3], in0=s3[:, 0:1],
                                         in1=s3[:, 0:1])
                else:
                    # ACT path: accum_out sums along the free axis.
                    scr = sb.tile([128, N], FP8, tag="gnscr", bufs=2)
                    nc.scalar.activation(
                        out=scr, in_=x_sb[ct],
                        func=mybir.ActivationFunctionType.Copy,
                        scale=1.0 / N, accum_out=s3[:, 0:1])
                    nc.scalar.activation(
                        out=scr, in_=x_sb[ct],
                        func=mybir.ActivationFunctionType.Square,
                        scale=1.0 / float(np.sqrt(N)), accum_out=s3[:, 1:2])
                    nc.vector.memset(s3[:, 2:3], 0.0)
                stats3.append(s3)

            ht = sb.tile([128, CT, N], FP8, name=f"h{s}", tag="h", bufs=2)
            for ct in range(CT):
                gp = ps.tile([GPT, 3], F32, tag="small", bufs=2)
                nc.tensor.matmul(gp, gmat, stats3[ct], start=True, stop=True)
                gs = sb.tile([GPT, 3], F32, tag="gs", bufs=4)
                nc.vector.tensor_copy(out=gs, in_=gp)
                # var_g = (E[var] + E[mean^2]) - (E[mean])^2
                m2 = sb.tile([GPT, 3], F32, tag="m2", bufs=4)
                nc.vector.tensor_add(out=m2[:, 1:2], in0=gs[:, 1:2],
                                     in1=gs[:, 2:3])
                nc.vector.tensor_mul(out=m2[:, 0:1], in0=gs[:, 0:1],
                                     in1=gs[:, 0:1])
                nc.vector.tensor_sub(out=m2[:, 2:3], in0=m2[:, 1:2],
                                     in1=m2[:, 0:1])
                s2 = sb.tile([GPT, 2], F32, tag="s2", bufs=4)
                nc.scalar.activation(out=s2[:, 1:2], in_=m2[:, 2:3],
                                     func=mybir.ActivationFunctionType.Sqrt,
                                     bias=eps_g, scale=1.0)
                nc.vector.tensor_copy(out=s2[:, 0:1], in_=gs[:, 0:1])
                nc.vector.reciprocal(out=s2[:, 1:2], in_=s2[:, 1:2])

                abp = ps.tile([128, 2], F32, tag="small", bufs=2)
                nc.tensor.matmul(abp, gmt, s2, start=True, stop=True)
                # a' = gnw' / sigma ; bneg' = a' mu - gnb'   (S_H pre-scaled)
                a_c = sb.tile([128, 1], F32, tag=f"a_{ct}", bufs=2)
                nc.vector.tensor_mul(out=a_c, in0=abp[:, 1:2],
                                     in1=b_sb["gnw"][ct])
                bneg = sb.tile([128, 1], F32, tag=f"bneg_{ct}", bufs=2)
                nc.vector.scalar_tensor_tensor(
                    out=bneg, in0=abp[:, 0:1], scalar=a_c,
                    in1=b_sb["gnb"][ct],
                    op0=mybir.AluOpType.mult, op1=mybir.AluOpType.subtract,
                )
                if ct < 2:
                    nc.vector.tensor_scalar(
                        out=ht[:, ct, :], in0=x_sb[ct], scalar1=a_c,
                        scalar2=bneg,
                        op0=mybir.AluOpType.mult,
                        op1=mybir.AluOpType.subtract,
                    )
                else:
                    # ACT path: Identity(a'*x + (-bneg')) — Identity lives in
                    # every activation table, so no table-load churn.
                    nbneg = sb.tile([128, 1], F32, tag=f"nb_{ct}", bufs=2)
                    nc.vector.tensor_scalar_mul(out=nbneg, in0=bneg,
                                                scalar1=-1.0)
                    nc.scalar.activation(
                        out=ht[:, ct, :], in_=x_sb[ct],
                        func=mybir.ActivationFunctionType.Identity,
                        scale=a_c, bias=nbneg)
            st[s]["h"] = ht

        def phase_tv(s):
            ht = st[s]["h"]
            # T[c2-slice, n] = sum_{c1-pairs} M~[:, pair, c2-slice].T @ h[:, pair, n]
            tt = sb.tile([128, CT, N], FP8, name=f"t{s}", tag="t", bufs=2)
            for ot in range(CT):
                for nch in range(NCH):
                    pp = ps.tile([128, NW], F32, tag="mm", bufs=6)
                    for cp in range(CT // 2):
                        nc.tensor.matmul(
                            pp,
                            mfus[:, 2 * cp:2 * cp + 2, ot * 128:(ot + 1) * 128],
                            ht[:, 2 * cp:2 * cp + 2, nch * NW:(nch + 1) * NW],
                            start=(cp == 0), stop=(cp == CT // 2 - 1),
                            perf_mode=DR)
                    nc.scalar.activation(
                        out=tt[:, ot, nch * NW:(nch + 1) * NW], in_=pp,
                        func=mybir.ActivationFunctionType.Copy, scale=T_EVAC)
            st[s]["t"] = tt
            # V2[m-slice, o] = sum_{c-pairs} h[:, pair, m-slice].T @ W2~[:, pair, :]
            v2 = sb.tile([128, MT, C], FP8, name=f"v2{s}", tag="v2", bufs=2)
            for mt in range(MT):
                vp = ps.tile([128, NW], F32, tag="mm", bufs=6)
                for cp in range(CT // 2):
                    nc.tensor.matmul(
                        vp,
                        ht[:, 2 * cp:2 * cp + 2, mt * 128:(mt + 1) * 128],
                        w2fus[:, 2 * cp:2 * cp + 2, :],
                        start=(cp == 0), stop=(cp == CT // 2 - 1),
                        perf_mode=DR)
                if mt % 2 == 0:
                    nc.vector.tensor_scalar_mul(out=v2[:, mt, :], in0=vp,
                                                scalar1=V2_EVAC)
                else:
                    nc.scalar.activation(
                        out=v2[:, mt, :], in_=vp,
                        func=mybir.ActivationFunctionType.Copy,
                        scale=V2_EVAC)
            st[s]["v2"] = v2
            # optional q/k-bias softmax term: wvec[m] = (S_h h)^T rvec
            if rvec is not None:
                ebias = sb.tile([128, MT], F32, name=f"eb{s}", tag="ebias",
                                bufs=2)
                for mt in range(MT):
                    wp = ps.tile([128, 1], F32, tag="wvec", bufs=2)
                    for cp in range(CT // 2):
                        nc.tensor.matmul(
                            wp,
                            ht[:, 2 * cp:2 * cp + 2, mt * 128:(mt + 1) * 128],
                            rvec[:, 2 * cp:2 * cp + 2, :],
                            start=(cp == 0), stop=(cp == CT // 2 - 1),
                            perf_mode=DR)
                    # bias[mt] = wvec_psum / (S_h * S_r * sqrt(C)) - EK
                    nc.vector.tensor_scalar(
                        out=ebias[:, mt:mt + 1], in0=wp,
                        scalar1=1.0 / (S_H * 256.0 * float(np.sqrt(C))),
                        scalar2=-EK,
                        op0=mybir.AluOpType.mult, op1=mybir.AluOpType.add)
                st[s]["ebias"] = ebias

        def phase_st(s):
            ht, tt = st[s]["h"], st[s]["t"]
            et = sb.tile([128, MT, N], FP8, name=f"e{s}", tag="e", bufs=2)
            eb = st[s].get("ebias")
            for nch in range(NCH):
                for mt in range(MT):
                    sp = ps.tile([128, NW], F32, tag="mm", bufs=6)
                    for cp in range(CT // 2):
                        nc.tensor.matmul(
                            sp,
                            ht[:, 2 * cp:2 * cp + 2, mt * 128:(mt + 1) * 128],
                            tt[:, 2 * cp:2 * cp + 2, nch * NW:(nch + 1) * NW],
                            start=(cp == 0), stop=(cp == CT // 2 - 1),
                            perf_mode=DR)
                    nc.scalar.activation(
                        out=et[:, mt, nch * NW:(nch + 1) * NW], in_=sp,
                        func=mybir.ActivationFunctionType.Exp,
                        scale=E_SCALE,
                        bias=(eb[:, mt:mt + 1] if eb is not None else nek),
                    )
            st[s]["e"] = et

        def phase_den(s):
            et = st[s]["e"]
            R_sb = sb.tile([128, N], F32, name=f"R{s}", tag="R", bufs=2)
            for nch in range(NCH):
                dp = ps.tile([128, NW], F32, tag="small", bufs=2)
                for mp in range(MT // 2):
                    nc.tensor.matmul(
                        dp, onesden,
                        et[:, 2 * mp:2 * mp + 2, nch * NW:(nch + 1) * NW],
                        start=(mp == 0), stop=(mp == MT // 2 - 1),
                        perf_mode=DR)
                # R = 1/(S_V2 * S_E * den): the S_V2 rides the ones value
                nc.vector.reciprocal(out=R_sb[:, nch * NW:(nch + 1) * NW],
                                     in_=dp)
            st[s]["R"] = R_sb

        def phase_av(s):
            x_sb, et, v2, R_sb = st[s]["x"], st[s]["e"], st[s]["v2"], st[s]["R"]
            for nch in range(NCH):
                for ot in range(CT):
                    op_ = ps.tile([128, NW], F32, tag="mm", bufs=6)
                    for mp in range(MT // 2):
                        nc.tensor.matmul(
                            op_,
                            v2[:, 2 * mp:2 * mp + 2, ot * 128:(ot + 1) * 128],
                            et[:, 2 * mp:2 * mp + 2, nch * NW:(nch + 1) * NW],
                            start=(mp == 0), stop=(mp == MT // 2 - 1),
                            perf_mode=DR)
                    tmp = sb.tile([128, NW], F32, tag="tmp", bufs=4)
                    nc.vector.tensor_mul(
                        out=tmp, in0=op_,
                        in1=R_sb[:, nch * NW:(nch + 1) * NW])
                    # y = (O + c0) + x, written in place over x
                    nc.vector.scalar_tensor_tensor(
                        out=x_sb[ot][:, nch * NW:(nch + 1) * NW], in0=tmp,
                        scalar=b_sb["c0"][ot],
                        in1=x_sb[ot][:, nch * NW:(nch + 1) * NW],
                        op0=mybir.AluOpType.add, op1=mybir.AluOpType.add,
                    )
                    nc.gpsimd.dma_start(
                        out=out_ext[s, ot, :, nch * NW:(nch + 1) * NW],
                        in_=x_sb[ot][:, nch * NW:(nch + 1) * NW])

        # x(s0) first (feeds GroupNorm), then weights, then x(s1).
        phase_load(0)
        phase_weights()
        phase_load(1)
        for phase in (phase_gn, phase_tv, phase_st, phase_den):
            for s in range(S):
                phase(s)
        for s in range(S):
            phase_av(s)


_CACHE = {}


def _q8(v, scale):
    import ml_dtypes
    return np.clip(np.asarray(v, np.float32) * scale, -240.0, 240.0).astype(
        ml_dtypes.float8_e4m3)


def make_in_maps(inputs):
    """Host-side weight folding + layout prep shared by kernel() and the
    test/sim harnesses. Returns (in_maps, has_qk_bias)."""
    x = np.asarray(inputs["x"], dtype=np.float32)
    assert x.shape == (B, C, H, W)

    wq = np.asarray(inputs["wq"], np.float64)
    wk = np.asarray(inputs["wk"], np.float64)
    wv = np.asarray(inputs["wv"], np.float64)
    wo = np.asarray(inputs["wo"], np.float64)
    bq = np.asarray(inputs["bq"], np.float64)
    bk = np.asarray(inputs["bk"], np.float64)

    # scores = h^T M h with M[c1,c2];  T[c2,n] = sum_c1 M[c1,c2] h[c1,n]
    M = wq.T @ wk
    # V2[m,o] = sum_c W2[o,c] h[c,m];  moving operand W2T[c,o]
    W2T = (wo @ wv).T
    mfus = np.ascontiguousarray(
        M.reshape(CT, 128, C).transpose(1, 0, 2))       # [128, ct(c1), c2]
    w2fus = np.ascontiguousarray(
        W2T.reshape(CT, 128, C).transpose(1, 0, 2))     # [128, ct(c), o]

    c0 = (wo @ np.asarray(inputs["bv"], np.float64)
          + np.asarray(inputs["bo"], np.float64)).astype(np.float32)

    gmat = np.zeros((128, GPT), dtype=np.float32)
    gmt = np.zeros((GPT, 128), dtype=np.float32)
    for g in range(GPT):
        gmat[g * GSIZE:(g + 1) * GSIZE, g] = 1.0 / GSIZE
        gmt[g, g * GSIZE:(g + 1) * GSIZE] = 1.0

    cblob = np.zeros((128, 20), dtype=np.float32)
    gnw = np.asarray(inputs["gn_weight"], np.float32) * S_H
    gnb = np.asarray(inputs["gn_bias"], np.float32) * S_H
    for bi, arr in enumerate((c0, gnw, gnb)):
        cblob[:, bi * CT:(bi + 1) * CT] = np.asarray(
            arr, dtype=np.float32).reshape(CT, 128).T
    cblob[:, 12:12 + GPT] = gmat

    base = {
        "mfus": _q8(mfus, S_M),
        "w2fus": _q8(w2fus, S_W2),
        "onesden": _q8(np.full((128, 256), S_V2, np.float32), 1.0),
        "cblob": cblob,
        "gmt": gmt,
    }

    has_qk_bias = bool(np.any(bq) or np.any(bk))
    if has_qk_bias:
        rv = (wk.T @ bq)                          # [C]; scale S_r = 256
        base["rvec"] = _q8(rv.reshape(CT, 128).T.reshape(128, CT, 1), 256.0)

    xr = x.reshape(NCORES, S, CT, 128, N)
    return ([dict(base, x=np.ascontiguousarray(xr[i])) for i in range(NCORES)],
            has_qk_bias)


def kernel(**inputs):
    in_maps, has_qk_bias = make_in_maps(inputs)
    key = ("nc", has_qk_bias)
    if key not in _CACHE:
        _CACHE[key] = build_nc(has_qk_bias=has_qk_bias)
    nc = _CACHE[key]

    res = run_bass_kernel_spmd(nc, in_maps, core_ids=list(range(NCORES)))

    out = np.empty((NCORES, S, CT, 128, N), dtype=np.float32)
    for i in range(NCORES):
        out[i] = res.results[i]["out"]
    return out.reshape(B, C, H, W)
